# revision 5
# baseline (speedup 1.0000x reference)
"""Multi-head causal attention (B=4, S=2048, D=1024, H=16) on 8 TRN2 cores.

Sharding: core c = (batch b = c//2, head-group g = c%2). Each core computes
8 heads of one batch end-to-end: QKV projections, causal flash attention,
and its half of the output projection (row-parallel Wo). Host sums the two
partial outputs per batch (the "all-reduce"); bias is added on device,
split half per core. Output partials are bf16 (summed in fp32 on host).

Device dataflow is fully transposed (xT in, outT out) so no on-device
transposes of activations are needed; V is transposed via one xbar DMA
transpose per pair (into a partition-tiled [128, NKB, 128] layout) plus
cheap DVE re-copies that insert the denominator ones column. All matmuls
are bf16 (fp32 PSUM accumulation) except a tiny f32r matmul that
broadcasts softmax reciprocals across partitions. Scores for the two
heads of a pair are issued back-to-back into disjoint PE row groups so
they run concurrently (contraction is only 64 deep). The causal structure
skips invalid 128x512 blocks entirely and trims the invalid left columns
of diagonal blocks from the scores/exp/mask/PV chain.

Scheduling: the next pair's QKV projection matmuls (and, on the last
pair, the output-projection matmuls) are pumped into the Tensor queue
between attention blocks so the PE stays busy while the Scalar engine
works through the exp chain. Softmax normalization happens per q-block
(reading the PV accumulator directly from PSUM) instead of per pair.
Input/weight DMAs are split across both HWDGE rings (sync + scalar) and
ordered so the first matmul can start as early as possible.
"""
import os
import sys
import types

import numpy as np
import ml_dtypes

from concourse import bacc, tile, bass_utils, mybir

B, S, D, H = 4, 2048, 1024, 16
HD = 64            # head dim
G = 2              # head groups (cores per batch)
DG = D // G        # 512 cols per core
NP = DG // 128     # 4 head-pairs per core
NCH = D // 128     # 8 contraction chunks
SB = 512           # q block
NSB = S // SB      # 4 q blocks
NKB = S // 128     # 16 k blocks

f32 = mybir.dt.float32
f32r = mybir.dt.float32r
bf16 = mybir.dt.bfloat16

LAST_RESULTS = None
_CACHE = {}


def _install_trace_shim():
    """Register the axon NTFF profile hook if this image's antenv lacks it."""
    if "antenv.axon_hooks" in sys.modules:
        return
    try:
        from trn_agent_boot.trn_boot import _ntff_profile_via_ctypes

        hook = _ntff_profile_via_ctypes("/opt/axon/libaxon_pjrt.so")
        mod = types.ModuleType("antenv.axon_hooks")
        mod.get_axon_ntff_profile_hook = lambda: hook
        mod.set_axon_ntff_profile_hook = lambda h: None
        sys.modules["antenv.axon_hooks"] = mod
        import antenv

        antenv.axon_hooks = mod
    except Exception:
        pass


def _build_program():
    nc = bacc.Bacc("TRN2", target_bir_lowering=False, debug=False)

    xT_d = nc.dram_tensor("xT", [D, S], bf16, kind="ExternalInput").ap()
    wq_d = nc.dram_tensor("Wq", [NP, 128, NCH, 128], bf16, kind="ExternalInput").ap()
    wk_d = nc.dram_tensor("Wk", [NP, 128, NCH, 128], bf16, kind="ExternalInput").ap()
    wv_d = nc.dram_tensor("Wv", [NP, 128, NCH, 128], bf16, kind="ExternalInput").ap()
    wo_d = nc.dram_tensor("Wo", [128, NP, NCH, 128], bf16, kind="ExternalInput").ap()
    bo_d = nc.dram_tensor("bo2", [128, NCH], f32, kind="ExternalInput").ap()
    mask_d = nc.dram_tensor("masks", [128, 4, SB], bf16, kind="ExternalInput").ap()
    sel_d = nc.dram_tensor("sel2", [33, 128], f32r, kind="ExternalInput").ap()
    out_d = nc.dram_tensor("outT", [D, S], bf16, kind="ExternalOutput").ap()

    xT_src = xT_d.rearrange("(c k) s -> k c s", k=128)
    out_dst = out_d.rearrange("(c k) s -> k c s", k=128)

    with tile.TileContext(nc) as tc:
        with (
            tc.tile_pool(name="const", bufs=1) as constp,
            tc.tile_pool(name="psc", bufs=2, space="PSUM") as psc,
            tc.tile_pool(name="psq", bufs=1, space="PSUM") as psq,
            tc.tile_pool(name="psy", bufs=2, space="PSUM") as psy,
            tc.tile_pool(name="ynormp", bufs=4) as ynormp,
            tc.tile_pool(name="outp", bufs=3) as outp,
        ):
            mask_sb = constp.tile([128, 4, SB], bf16)
            sel_sb = constp.tile([33, 128], f32r)
            bo_sb = constp.tile([128, NCH], f32)
            wo_sb = constp.tile([128, NP, NCH, 128], bf16)
            # scalar HWDGE ring: constants; wo (1 MB, needed only at the
            # end) goes last so it never delays the compute-critical DMAs.
            nc.scalar.dma_start(mask_sb[:], mask_d[:])
            nc.scalar.dma_start(sel_sb[:], sel_d[:])
            nc.scalar.dma_start(bo_sb[:], bo_d[:])
            nc.scalar.dma_start(wo_sb[:], wo_d[:])

            ynorm = []  # per-pair [128, S] bf16 normalized attention outputs

            with (
                tc.tile_pool(name="xtp", bufs=1) as xtp,
                tc.tile_pool(name="wp", bufs=2) as wp,
                tc.tile_pool(name="qkv", bufs=2) as qkvp,
                tc.tile_pool(name="vtp", bufs=2) as vtp,
                tc.tile_pool(name="vstp", bufs=2) as vstp,
                tc.tile_pool(name="vp", bufs=2) as vpool,
                tc.tile_pool(name="pp", bufs=4) as ppool,
                tc.tile_pool(name="yun", bufs=4) as yunp,
                tc.tile_pool(name="rp", bufs=2) as rpool,
            ):
                xt = xtp.tile([128, NCH, S], bf16)

                def alloc_pair(pp_):
                    qt_ = qkvp.tile([128, S], bf16, tag="qt", name="qt")
                    kt_ = qkvp.tile([128, S], bf16, tag="kt", name="kt")
                    vt_ = vtp.tile([128, S], bf16, tag="vt", name="vt")
                    # per (kb, h): 64 head dims + ones col at 64 (+1 pad
                    # so the h stride is 4B-aligned for fast DVE copies)
                    vs_ = vpool.tile([128, NKB, 2, 66], bf16, tag="v",
                                     name="v_sb")
                    vst_ = vstp.tile([128, NKB, 128], bf16, tag="vst",
                                     name="vst")
                    nc.vector.memset(vs_[:, :, :, 64:65], 1.0)
                    return qt_, kt_, vt_, vs_, vst_

                def emit_qkv(pp_, qt_, kt_, vt_, vs_, vst_):
                    """Yield small units of pair pp_'s QKV projection work.

                    V is projected first so its transpose (one xbar DMA
                    for the whole [128, S] tile) can start early.
                    """
                    w_tiles = []
                    for nm, wd in (("wv", wv_d), ("wq", wq_d), ("wk", wk_d)):
                        wt = wp.tile([128, NCH, 128], bf16, tag=nm, name=nm)
                        nc.sync.dma_start(wt[:], wd[pp_])
                        w_tiles.append(wt)
                    yield
                    for wi, (wt, dst) in enumerate(
                            zip(w_tiles, (vt_, qt_, kt_))):
                        for g2 in range(2):
                            acc = psq.tile([128, 2, SB], f32, tag="qacc",
                                           name="qacc")
                            for si in range(2):
                                sblk = 2 * g2 + si
                                for ci in range(NCH):
                                    nc.tensor.matmul(
                                        acc[:, si, :],
                                        wt[:, ci, :],
                                        xt[:, ci, sblk * SB:(sblk + 1) * SB],
                                        start=(ci == 0),
                                        stop=(ci == NCH - 1),
                                    )
                                    if ci % 4 == 3:
                                        yield
                            nc.vector.tensor_copy(
                                dst[:, 2 * g2 * SB:2 * (g2 + 1) * SB],
                                acc[:],
                            )
                            yield
                        if wi == 0:
                            # vt complete: one xbar transpose VT -> V
                            # (s = kb*128 + partition tiling), then per-kb
                            # DVE copies into the ones-augmented layout.
                            nc.sync.dma_start_transpose(vst_[:], vt_[:])
                            for kb in range(NKB):
                                nc.vector.tensor_copy(
                                    vs_[:, kb, :, 0:64],
                                    vst_[:, kb, :].rearrange(
                                        "k (h d) -> k h d", h=2),
                                )
                                if kb % 4 == 3:
                                    yield

                def emit_outproj(j):
                    """outT[:, j block] = Wo_g.T @ ynorm (+ bo/2), bf16."""
                    for g2 in range(4):
                        acc = psq.tile([128, 2, SB], f32, tag="qacc",
                                       name="oacc")
                        for pp in range(NP):
                            for si in range(2):
                                dc = 2 * g2 + si
                                nc.tensor.matmul(
                                    acc[:, si, :],
                                    wo_sb[:, pp, dc, :],
                                    ynorm[pp][:, j * SB:(j + 1) * SB],
                                    start=(pp == 0),
                                    stop=(pp == NP - 1),
                                )
                            yield
                        ot = outp.tile([128, 2, SB], bf16, tag="ot",
                                       name="ot")
                        for si in range(2):
                            dc = 2 * g2 + si
                            nc.vector.tensor_scalar_add(
                                ot[:, si, :],
                                acc[:, si, :],
                                bo_sb[:, dc:dc + 1],
                            )
                        nc.sync.dma_start(
                            out_dst[:, 2 * g2:2 * g2 + 2,
                                    j * SB:(j + 1) * SB],
                            ot[:],
                        )
                        yield

                pending = []

                def pump(n=1):
                    for _ in range(n):
                        while pending:
                            try:
                                next(pending[0])
                                break
                            except StopIteration:
                                pending.pop(0)
                        else:
                            break

                def drain():
                    while pending:
                        try:
                            next(pending[0])
                        except StopIteration:
                            pending.pop(0)

                # prologue: pair-0 weights on the sync ring first, then x
                cur = alloc_pair(0)
                g0 = emit_qkv(0, *cur)
                next(g0)  # issues the three weight DMAs
                for qtr in range(4):
                    nc.sync.dma_start(
                        xt[:, :, qtr * SB:(qtr + 1) * SB],
                        xT_src[:, :, qtr * SB:(qtr + 1) * SB],
                    )
                for _ in g0:
                    pass

                for p in range(NP):
                    qt, kt, vt, vs_, vst_ = cur
                    if p + 1 < NP:
                        nxt = alloc_pair(p + 1)
                        pending.append(emit_qkv(p + 1, *nxt))
                    else:
                        nxt = None
                    yn = ynormp.tile([128, S], bf16, tag="yn", name="yn")
                    ynorm.append(yn)

                    # ---- causal flash attention, both heads interleaved --
                    for j in range(NSB):
                        nkb_j = 4 * (j + 1)
                        yaccs = [
                            psy.tile([128, SB], f32, tag="y", name="yacc")
                            for _ in range(2)
                        ]
                        pend = None
                        for kb in range(nkb_j):
                            d = kb - 4 * j
                            qlo = max(0, d) * 128  # causal column trim
                            sc = psc.tile([128, 2, SB], f32, tag="sc",
                                          name="sc")
                            for h in range(2):
                                hlo, hhi = h * 64, (h + 1) * 64
                                nc.tensor.matmul(
                                    sc[:, h, qlo:],
                                    kt[hlo:hhi, kb * 128:(kb + 1) * 128],
                                    qt[hlo:hhi, j * SB + qlo:(j + 1) * SB],
                                    start=True,
                                    stop=True,
                                )
                            pt = ppool.tile([128, 2, SB], bf16, tag="p",
                                            name="pt")
                            nc.scalar.activation(
                                pt[:, :, qlo:],
                                sc[:, :, qlo:],
                                mybir.ActivationFunctionType.Exp,
                                scale=0.125,
                            )
                            if d >= 0:
                                for h in range(2):
                                    nc.vector.tensor_mul(
                                        pt[:, h, qlo:],
                                        pt[:, h, qlo:],
                                        mask_sb[:, d, qlo:],
                                    )
                            if pend is not None:  # y lags one block behind
                                kb_, pt_, qlo_ = pend
                                for h in range(2):
                                    nc.tensor.matmul(
                                        yaccs[h][0:65, qlo_:],
                                        vs_[:, kb_, h, 0:65],
                                        pt_[:, h, qlo_:],
                                        start=(kb_ == 0),
                                        stop=False,
                                    )
                            pend = (kb, pt, qlo)
                            pump()
                        kb_, pt_, qlo_ = pend
                        for h in range(2):
                            nc.tensor.matmul(
                                yaccs[h][0:65, qlo_:],
                                vs_[:, kb_, h, 0:65],
                                pt_[:, h, qlo_:],
                                start=False,
                                stop=True,
                            )
                        # ---- per-j normalize: yn = yacc[0:64] / yacc[64] --
                        y_uns = []
                        for h in range(2):
                            y_un = yunp.tile([64, SB], f32, tag="yun",
                                             name="y_un")
                            nc.vector.tensor_copy(y_un[:], yaccs[h][0:64, :])
                            y_uns.append(y_un)
                        d2 = rpool.tile([33, SB], f32, tag="d", name="d2")
                        r2 = rpool.tile([33, SB], f32, tag="r", name="r2")
                        rr = rpool.tile([33, SB], f32r, tag="rr", name="rr")
                        if p == 0 and j < 2:
                            # first use of each rotating buffer: clear the
                            # unused rows so the recip/f32r cast/matmul
                            # never touch NaN garbage
                            nc.vector.memset(d2[:], 1.0)
                        for h in range(2):
                            # cross-partition move must be a tensor_copy
                            nc.vector.tensor_copy(
                                d2[32 * h:32 * h + 1, :],
                                yaccs[h][64:65, :])
                        nc.vector.reciprocal_approx_fast(r2[:], d2[:])
                        nc.vector.tensor_copy(rr[:], r2[:])
                        bc = psy.tile([128, SB], f32, tag="y", name="bc")
                        nc.tensor.matmul(
                            bc[:],
                            sel_sb[:],
                            rr[:],
                            start=True,
                            stop=True,
                        )
                        for h in range(2):
                            hlo, hhi = h * 64, (h + 1) * 64
                            nc.vector.tensor_mul(
                                yn[hlo:hhi, j * SB:(j + 1) * SB],
                                y_uns[h][:],
                                bc[hlo:hhi, :],
                            )
                        pump()
                        if p == NP - 1:
                            pending.append(emit_outproj(j))
                            pump()
                    drain()
                    cur = nxt

    nc.compile()
    return nc


def _get_program():
    if "nc" not in _CACHE:
        _CACHE["nc"] = _build_program()
    return _CACHE["nc"]


def kernel(x, Wq, Wk, Wv, Wo, bo):
    global LAST_RESULTS
    x = np.asarray(x, dtype=np.float32)
    Wq = np.asarray(Wq, dtype=np.float32)
    Wk = np.asarray(Wk, dtype=np.float32)
    Wv = np.asarray(Wv, dtype=np.float32)
    Wo = np.asarray(Wo, dtype=np.float32)
    bo = np.asarray(bo, dtype=np.float32)

    nc = _get_program()

    # constants shared by all cores
    masks = np.zeros((128, 4, SB), dtype=ml_dtypes.bfloat16)
    kk = np.arange(128)[:, None]
    qq = np.arange(SB)[None, :]
    for d in range(4):
        masks[:, d, :] = (128 * d + kk <= qq).astype(ml_dtypes.bfloat16)
    sel2 = np.zeros((33, 128), dtype=np.float32)
    sel2[0, 0:64] = 1.0
    sel2[32, 64:128] = 1.0
    bo2 = np.ascontiguousarray((bo / 2.0).reshape(NCH, 128).T)

    def wshape(w):  # [D, DG] -> [NP, 128, NCH, 128] bf16
        return np.ascontiguousarray(
            w.reshape(NCH, 128, NP, 128).transpose(2, 1, 0, 3)
        ).astype(ml_dtypes.bfloat16)

    in_maps = []
    for c in range(8):
        b, g = c // 2, c % 2
        xT = np.ascontiguousarray(x[b].T).astype(ml_dtypes.bfloat16)
        wo_g = Wo[g * DG:(g + 1) * DG, :]
        wo_dev = np.ascontiguousarray(
            wo_g.reshape(NP, 128, NCH, 128).transpose(1, 0, 2, 3)
        ).astype(ml_dtypes.bfloat16)
        in_maps.append({
            "xT": xT,
            "Wq": wshape(Wq[:, g * DG:(g + 1) * DG]),
            "Wk": wshape(Wk[:, g * DG:(g + 1) * DG]),
            "Wv": wshape(Wv[:, g * DG:(g + 1) * DG]),
            "Wo": wo_dev,
            "bo2": bo2,
            "masks": masks,
            "sel2": sel2,
        })

    trace = bool(os.environ.get("BASS_TRACE"))
    if trace:
        _install_trace_shim()
    res = None
    for attempt in range(3):
        try:
            res = bass_utils.run_bass_kernel_spmd(
                nc, in_maps, core_ids=list(range(8)), trace=trace)
            break
        except Exception:
            if attempt == 2:
                raise
    LAST_RESULTS = res

    out = np.empty((B, S, D), dtype=np.float32)
    for b in range(B):
        acc = (res.results[2 * b]["outT"].astype(np.float32)
               + res.results[2 * b + 1]["outT"].astype(np.float32))
        out[b] = acc.T
    return out


# revision 8
# speedup vs baseline: 1.1101x; 1.1101x over previous
"""Multi-head causal attention (B=4, S=2048, D=1024, H=16) on 8 TRN2 cores.

Sharding: core c = (batch b = c//2, head-group g = c%2). Each core computes
8 heads of one batch end-to-end: QKV projections, causal flash attention,
and its half of the output projection (row-parallel Wo). Host sums the two
partial outputs per batch (the "all-reduce"); bias is added on device,
split half per core. Output partials are bf16 (summed in fp32 on host).

Device dataflow is fully transposed (xT in, outT out) so no on-device
transposes of activations are needed; V is transposed via one xbar DMA
transpose per pair (into a partition-tiled [128, NKB, 128] layout) plus
cheap DVE re-copies that insert the denominator ones column. All matmuls
are bf16 (fp32 PSUM accumulation) except a tiny f32r matmul that
broadcasts softmax reciprocals across partitions. Scores for the two
heads of a pair are issued back-to-back into disjoint PE row groups so
they run concurrently (contraction is only 64 deep). The causal structure
skips invalid 128x512 blocks entirely and trims the invalid left columns
of diagonal blocks from the scores/exp/mask/PV chain.

Scheduling: the next pair's QKV projection matmuls (and, on the last
pair, the output-projection matmuls) are pumped into the Tensor queue
between attention blocks so the PE stays busy while the Scalar engine
works through the exp chain. Softmax normalization happens per q-block
(reading the PV accumulator directly from PSUM) instead of per pair.
Input/weight DMAs are split across both HWDGE rings (sync + scalar) and
ordered so the first matmul can start as early as possible.
"""
import os
import sys
import types

import numpy as np
import ml_dtypes

from concourse import bacc, tile, bass_utils, mybir

B, S, D, H = 4, 2048, 1024, 16
HD = 64            # head dim
G = 2              # head groups (cores per batch)
DG = D // G        # 512 cols per core
NP = DG // 128     # 4 head-pairs per core
NCH = D // 128     # 8 contraction chunks
SB = 512           # q block
NSB = S // SB      # 4 q blocks
NKB = S // 128     # 16 k blocks

f32 = mybir.dt.float32
f32r = mybir.dt.float32r
bf16 = mybir.dt.bfloat16

LAST_RESULTS = None
_CACHE = {}


def _install_trace_shim():
    """Register the axon NTFF profile hook if this image's antenv lacks it."""
    if "antenv.axon_hooks" in sys.modules:
        return
    try:
        from trn_agent_boot.trn_boot import _ntff_profile_via_ctypes

        hook = _ntff_profile_via_ctypes("/opt/axon/libaxon_pjrt.so")
        mod = types.ModuleType("antenv.axon_hooks")
        mod.get_axon_ntff_profile_hook = lambda: hook
        mod.set_axon_ntff_profile_hook = lambda h: None
        sys.modules["antenv.axon_hooks"] = mod
        import antenv

        antenv.axon_hooks = mod
    except Exception:
        pass


def _build_program():
    nc = bacc.Bacc("TRN2", target_bir_lowering=False, debug=False)

    xT_d = nc.dram_tensor("xT", [D, S], bf16, kind="ExternalInput").ap()
    wq_d = nc.dram_tensor("Wq", [NP, 128, NCH, 128], bf16, kind="ExternalInput").ap()
    wk_d = nc.dram_tensor("Wk", [NP, 128, NCH, 128], bf16, kind="ExternalInput").ap()
    wv_d = nc.dram_tensor("Wv", [NP, 128, NCH, 128], bf16, kind="ExternalInput").ap()
    wo_d = nc.dram_tensor("Wo", [128, NP, NCH, 128], bf16, kind="ExternalInput").ap()
    bo_d = nc.dram_tensor("bo2", [128, NCH], f32, kind="ExternalInput").ap()
    mask_d = nc.dram_tensor("masks", [128, 4, SB], bf16, kind="ExternalInput").ap()
    sel_d = nc.dram_tensor("sel2", [33, 128], f32r, kind="ExternalInput").ap()
    out_d = nc.dram_tensor("outT", [D, S], bf16, kind="ExternalOutput").ap()

    xT_src = xT_d.rearrange("(c k) s -> k c s", k=128)
    out_dst = out_d.rearrange("(c k) s -> k c s", k=128)

    with tile.TileContext(nc) as tc:
        with (
            tc.tile_pool(name="const", bufs=1) as constp,
            tc.tile_pool(name="psc", bufs=2, space="PSUM") as psc,
            tc.tile_pool(name="psq", bufs=2, space="PSUM") as psq,
            tc.tile_pool(name="psy", bufs=2, space="PSUM") as psy,
            tc.tile_pool(name="ynormp", bufs=4) as ynormp,
            tc.tile_pool(name="outp", bufs=3) as outp,
        ):
            mask_sb = constp.tile([128, 4, SB], bf16)
            sel_sb = constp.tile([33, 128], f32r)
            bo_sb = constp.tile([128, NCH], f32)
            wo_sb = constp.tile([128, NP, NCH, 128], bf16)
            # scalar HWDGE ring: constants; wo (1 MB, needed only at the
            # end) goes last so it never delays the compute-critical DMAs.
            nc.scalar.dma_start(mask_sb[:], mask_d[:])
            nc.scalar.dma_start(sel_sb[:], sel_d[:])
            nc.scalar.dma_start(bo_sb[:], bo_d[:])
            nc.scalar.dma_start(wo_sb[:], wo_d[:])

            ynorm = []  # per-pair [128, S] bf16 normalized attention outputs

            with (
                tc.tile_pool(name="xtp", bufs=1) as xtp,
                tc.tile_pool(name="wp", bufs=2) as wp,
                tc.tile_pool(name="qkv", bufs=2) as qkvp,
                tc.tile_pool(name="vtp", bufs=2) as vtp,
                tc.tile_pool(name="vstp", bufs=2) as vstp,
                tc.tile_pool(name="vp", bufs=2) as vpool,
                tc.tile_pool(name="pp", bufs=4) as ppool,
                tc.tile_pool(name="yun", bufs=4) as yunp,
                tc.tile_pool(name="rp", bufs=2) as rpool,
            ):
                xt = xtp.tile([128, NCH, S], bf16)

                def alloc_pair(pp_):
                    qt_ = qkvp.tile([128, S], bf16, tag="qt", name="qt")
                    kt_ = qkvp.tile([128, S], bf16, tag="kt", name="kt")
                    vt_ = vtp.tile([128, S], bf16, tag="vt", name="vt")
                    # per (kb, h): 64 head dims + ones col at 64 (+1 pad
                    # so the h stride is 4B-aligned for fast DVE copies)
                    vs_ = vpool.tile([128, NKB, 2, 66], bf16, tag="v",
                                     name="v_sb")
                    vst_ = vstp.tile([128, NKB, 128], bf16, tag="vst",
                                     name="vst")
                    nc.vector.memset(vs_[:, :, :, 64:65], 1.0)
                    return qt_, kt_, vt_, vs_, vst_

                def emit_qkv(pp_, qt_, kt_, vt_, vs_, vst_):
                    """Yield small units of pair pp_'s QKV projection work.

                    V is projected first so its transpose (one xbar DMA
                    for the whole [128, S] tile) can start early.
                    """
                    w_tiles = []
                    for nm, wd in (("wv", wv_d), ("wq", wq_d), ("wk", wk_d)):
                        wt = wp.tile([128, NCH, 128], bf16, tag=nm, name=nm)
                        nc.sync.dma_start(wt[:], wd[pp_])
                        w_tiles.append(wt)
                    yield
                    for wi, (wt, dst) in enumerate(
                            zip(w_tiles, (vt_, qt_, kt_))):
                        for sblk in range(4):
                            acc = psq.tile([128, SB], f32, tag="acc",
                                           name="qacc")
                            for ci in range(NCH):
                                nc.tensor.matmul(
                                    acc[:],
                                    wt[:, ci, :],
                                    xt[:, ci, sblk * SB:(sblk + 1) * SB],
                                    start=(ci == 0),
                                    stop=(ci == NCH - 1),
                                )
                                if ci % 4 == 3:
                                    yield
                            nc.vector.tensor_copy(
                                dst[:, sblk * SB:(sblk + 1) * SB],
                                acc[:],
                            )
                            yield
                        if wi == 0:
                            # vt complete: one xbar transpose VT -> V
                            # (s = kb*128 + partition tiling), then per-kb
                            # DVE copies into the ones-augmented layout.
                            nc.sync.dma_start_transpose(vst_[:], vt_[:])
                            for kb in range(NKB):
                                nc.vector.tensor_copy(
                                    vs_[:, kb, :, 0:64],
                                    vst_[:, kb, :].rearrange(
                                        "k (h d) -> k h d", h=2),
                                )
                                if kb % 4 == 3:
                                    yield

                def emit_outproj(j):
                    """outT[:, j block] = Wo_g.T @ ynorm (+ bo/2), bf16."""
                    for g2 in range(4):
                        ot = outp.tile([128, 2, SB], bf16, tag="ot",
                                       name="ot")
                        for si in range(2):
                            dc = 2 * g2 + si
                            acc = psq.tile([128, SB], f32, tag="acc",
                                           name="oacc")
                            for pp in range(NP):
                                nc.tensor.matmul(
                                    acc[:],
                                    wo_sb[:, pp, dc, :],
                                    ynorm[pp][:, j * SB:(j + 1) * SB],
                                    start=(pp == 0),
                                    stop=(pp == NP - 1),
                                )
                                if pp % 2 == 1:
                                    yield
                            nc.vector.tensor_scalar_add(
                                ot[:, si, :],
                                acc[:],
                                bo_sb[:, dc:dc + 1],
                            )
                        nc.sync.dma_start(
                            out_dst[:, 2 * g2:2 * g2 + 2,
                                    j * SB:(j + 1) * SB],
                            ot[:],
                        )
                        yield

                pending = []

                def pump(n=1):
                    for _ in range(n):
                        while pending:
                            try:
                                next(pending[0])
                                break
                            except StopIteration:
                                pending.pop(0)
                        else:
                            break

                def drain():
                    while pending:
                        try:
                            next(pending[0])
                        except StopIteration:
                            pending.pop(0)

                # prologue: pair-0 weights on the sync ring first, then x
                cur = alloc_pair(0)
                g0 = emit_qkv(0, *cur)
                next(g0)  # issues the three weight DMAs
                for qtr in range(4):
                    eng = nc.sync if qtr % 2 == 0 else nc.scalar
                    eng.dma_start(
                        xt[:, :, qtr * SB:(qtr + 1) * SB],
                        xT_src[:, :, qtr * SB:(qtr + 1) * SB],
                    )
                for _ in g0:
                    pass

                for p in range(NP):
                    qt, kt, vt, vs_, vst_ = cur
                    if p + 1 < NP:
                        nxt = alloc_pair(p + 1)
                        pending.append(emit_qkv(p + 1, *nxt))
                    else:
                        nxt = None
                    yn = ynormp.tile([128, S], bf16, tag="yn", name="yn")
                    ynorm.append(yn)

                    # ---- causal flash attention, both heads interleaved --
                    for j in range(NSB):
                        nkb_j = 4 * (j + 1)
                        yaccs = [
                            psy.tile([128, SB], f32, tag="y", name="yacc")
                            for _ in range(2)
                        ]
                        pend = None
                        for kb in range(nkb_j):
                            d = kb - 4 * j
                            qlo = max(0, d) * 128  # causal column trim
                            sc = psc.tile([128, 2, SB], f32, tag="sc",
                                          name="sc")
                            for h in range(2):
                                hlo, hhi = h * 64, (h + 1) * 64
                                nc.tensor.matmul(
                                    sc[:, h, qlo:],
                                    kt[hlo:hhi, kb * 128:(kb + 1) * 128],
                                    qt[hlo:hhi, j * SB + qlo:(j + 1) * SB],
                                    start=True,
                                    stop=True,
                                )
                            pt = ppool.tile([128, 2, SB], bf16, tag="p",
                                            name="pt")
                            nc.scalar.activation(
                                pt[:, :, qlo:],
                                sc[:, :, qlo:],
                                mybir.ActivationFunctionType.Exp,
                                scale=0.125,
                            )
                            if d >= 0:
                                for h in range(2):
                                    nc.vector.tensor_mul(
                                        pt[:, h, qlo:],
                                        pt[:, h, qlo:],
                                        mask_sb[:, d, qlo:],
                                    )
                            if pend is not None:  # y lags one block behind
                                kb_, pt_, qlo_ = pend
                                for h in range(2):
                                    nc.tensor.matmul(
                                        yaccs[h][0:65, qlo_:],
                                        vs_[:, kb_, h, 0:65],
                                        pt_[:, h, qlo_:],
                                        start=(kb_ == 0),
                                        stop=False,
                                    )
                            pend = (kb, pt, qlo)
                            pump(2 if p == NP - 1 else 1)
                        kb_, pt_, qlo_ = pend
                        for h in range(2):
                            nc.tensor.matmul(
                                yaccs[h][0:65, qlo_:],
                                vs_[:, kb_, h, 0:65],
                                pt_[:, h, qlo_:],
                                start=False,
                                stop=True,
                            )
                        # ---- per-j normalize: yn = yacc[0:64] / yacc[64] --
                        y_uns = []
                        for h in range(2):
                            y_un = yunp.tile([65, SB], f32, tag="yun",
                                             name="y_un")
                            nc.vector.tensor_copy(y_un[:], yaccs[h][0:65, :])
                            y_uns.append(y_un)
                        d2 = rpool.tile([33, SB], f32, tag="d", name="d2")
                        r2 = rpool.tile([33, SB], f32, tag="r", name="r2")
                        rr = rpool.tile([33, SB], f32r, tag="rr", name="rr")
                        if p == 0 and j < 2:
                            # first use of each rotating buffer: clear the
                            # unused rows so the recip/f32r cast/matmul
                            # never touch NaN garbage
                            nc.vector.memset(d2[:], 1.0)
                        for h in range(2):
                            # cross-partition move must be a tensor_copy
                            nc.vector.tensor_copy(
                                d2[32 * h:32 * h + 1, :],
                                y_uns[h][64:65, :])
                        nc.vector.reciprocal_approx_fast(r2[:], d2[:])
                        nc.vector.tensor_copy(rr[:], r2[:])
                        bc = psy.tile([128, SB], f32, tag="y", name="bc")
                        nc.tensor.matmul(
                            bc[:],
                            sel_sb[:],
                            rr[:],
                            start=True,
                            stop=True,
                        )
                        for h in range(2):
                            hlo, hhi = h * 64, (h + 1) * 64
                            nc.vector.tensor_mul(
                                yn[hlo:hhi, j * SB:(j + 1) * SB],
                                y_uns[h][0:64, :],
                                bc[hlo:hhi, :],
                            )
                        pump()
                        if p == NP - 1:
                            pending.append(emit_outproj(j))
                            pump()
                    drain()
                    cur = nxt

    nc.compile()
    return nc


def _get_program():
    if "nc" not in _CACHE:
        _CACHE["nc"] = _build_program()
    return _CACHE["nc"]


def kernel(x, Wq, Wk, Wv, Wo, bo):
    global LAST_RESULTS
    x = np.asarray(x, dtype=np.float32)
    Wq = np.asarray(Wq, dtype=np.float32)
    Wk = np.asarray(Wk, dtype=np.float32)
    Wv = np.asarray(Wv, dtype=np.float32)
    Wo = np.asarray(Wo, dtype=np.float32)
    bo = np.asarray(bo, dtype=np.float32)

    nc = _get_program()

    # constants shared by all cores
    masks = np.zeros((128, 4, SB), dtype=ml_dtypes.bfloat16)
    kk = np.arange(128)[:, None]
    qq = np.arange(SB)[None, :]
    for d in range(4):
        masks[:, d, :] = (128 * d + kk <= qq).astype(ml_dtypes.bfloat16)
    sel2 = np.zeros((33, 128), dtype=np.float32)
    sel2[0, 0:64] = 1.0
    sel2[32, 64:128] = 1.0
    bo2 = np.ascontiguousarray((bo / 2.0).reshape(NCH, 128).T)

    def wshape(w):  # [D, DG] -> [NP, 128, NCH, 128] bf16
        return np.ascontiguousarray(
            w.reshape(NCH, 128, NP, 128).transpose(2, 1, 0, 3)
        ).astype(ml_dtypes.bfloat16)

    in_maps = []
    for c in range(8):
        b, g = c // 2, c % 2
        xT = np.ascontiguousarray(x[b].T).astype(ml_dtypes.bfloat16)
        wo_g = Wo[g * DG:(g + 1) * DG, :]
        wo_dev = np.ascontiguousarray(
            wo_g.reshape(NP, 128, NCH, 128).transpose(1, 0, 2, 3)
        ).astype(ml_dtypes.bfloat16)
        in_maps.append({
            "xT": xT,
            "Wq": wshape(Wq[:, g * DG:(g + 1) * DG]),
            "Wk": wshape(Wk[:, g * DG:(g + 1) * DG]),
            "Wv": wshape(Wv[:, g * DG:(g + 1) * DG]),
            "Wo": wo_dev,
            "bo2": bo2,
            "masks": masks,
            "sel2": sel2,
        })

    trace = bool(os.environ.get("BASS_TRACE"))
    if trace:
        _install_trace_shim()
    res = None
    for attempt in range(3):
        try:
            res = bass_utils.run_bass_kernel_spmd(
                nc, in_maps, core_ids=list(range(8)), trace=trace)
            break
        except Exception:
            if attempt == 2:
                raise
    LAST_RESULTS = res

    out = np.empty((B, S, D), dtype=np.float32)
    for b in range(B):
        acc = (res.results[2 * b]["outT"].astype(np.float32)
               + res.results[2 * b + 1]["outT"].astype(np.float32))
        out[b] = acc.T
    return out


# revision 12
# speedup vs baseline: 1.1997x; 1.0807x over previous
"""Multi-head causal attention (B=4, S=2048, D=1024, H=16) on 8 TRN2 cores.

Sharding: core c = (batch b = c//2, head-group g = c%2). Each core computes
8 heads of one batch end-to-end: QKV projections, causal flash attention,
and its half of the output projection (row-parallel Wo). Host sums the two
partial outputs per batch (the "all-reduce"); bias is added on device,
split half per core. Output partials are bf16 (summed in fp32 on host).

Device dataflow is fully transposed (xT in, outT out) so no on-device
transposes of activations are needed; V is transposed via one xbar DMA
transpose per pair (into a partition-tiled [128, NKB, 128] layout) plus
cheap DVE re-copies that insert the denominator ones column. All matmuls
are bf16 (fp32 PSUM accumulation) except a tiny f32r matmul that
broadcasts softmax reciprocals across partitions. Scores for the two
heads of a pair are issued back-to-back into disjoint PE row groups so
they run concurrently (contraction is only 64 deep). The causal structure
skips invalid 128x512 blocks entirely and trims the invalid left columns
of diagonal blocks from the scores/exp/mask/PV chain.

Scheduling: the next pair's QKV projection matmuls (and, on the last
pair, the output-projection matmuls) are pumped into the Tensor queue
between attention blocks so the PE stays busy while the Scalar engine
works through the exp chain. Softmax normalization happens per q-block
(reading the PV accumulator directly from PSUM) instead of per pair.
Input/weight DMAs are split across both HWDGE rings (sync + scalar) and
ordered so the first matmul can start as early as possible.
"""
import os
import sys
import types

import numpy as np
import ml_dtypes

from concourse import bacc, tile, bass_utils, mybir

B, S, D, H = 4, 2048, 1024, 16
HD = 64            # head dim
G = 2              # head groups (cores per batch)
DG = D // G        # 512 cols per core
NP = DG // 128     # 4 head-pairs per core
NCH = D // 128     # 8 contraction chunks
SB = 512           # q block
NSB = S // SB      # 4 q blocks
NKB = S // 128     # 16 k blocks

f32 = mybir.dt.float32
f32r = mybir.dt.float32r
bf16 = mybir.dt.bfloat16

LAST_RESULTS = None
_CACHE = {}


def _install_trace_shim():
    """Register the axon NTFF profile hook if this image's antenv lacks it."""
    if "antenv.axon_hooks" in sys.modules:
        return
    try:
        from trn_agent_boot.trn_boot import _ntff_profile_via_ctypes

        hook = _ntff_profile_via_ctypes("/opt/axon/libaxon_pjrt.so")
        mod = types.ModuleType("antenv.axon_hooks")
        mod.get_axon_ntff_profile_hook = lambda: hook
        mod.set_axon_ntff_profile_hook = lambda h: None
        sys.modules["antenv.axon_hooks"] = mod
        import antenv

        antenv.axon_hooks = mod
    except Exception:
        pass


def _build_program():
    nc = bacc.Bacc("TRN2", target_bir_lowering=False, debug=False)

    xT_d = nc.dram_tensor("xT", [D, S], bf16, kind="ExternalInput").ap()
    wq_d = nc.dram_tensor("Wq", [NP, 128, NCH, 128], bf16, kind="ExternalInput").ap()
    wk_d = nc.dram_tensor("Wk", [NP, 128, NCH, 128], bf16, kind="ExternalInput").ap()
    wv_d = nc.dram_tensor("Wv", [NP, 128, NCH, 128], bf16, kind="ExternalInput").ap()
    wo_d = nc.dram_tensor("Wo", [128, NP, NCH, 128], bf16, kind="ExternalInput").ap()
    bo_d = nc.dram_tensor("bo2", [128, NCH], f32, kind="ExternalInput").ap()
    mask_d = nc.dram_tensor("masks", [128, 4, SB], bf16, kind="ExternalInput").ap()
    out_d = nc.dram_tensor("outT", [D, S], bf16, kind="ExternalOutput").ap()

    xT_src = xT_d.rearrange("(c k) s -> k c s", k=128)
    out_dst = out_d.rearrange("(c k) s -> k c s", k=128)

    with tile.TileContext(nc) as tc:
        with (
            tc.tile_pool(name="const", bufs=1) as constp,
            tc.tile_pool(name="psc", bufs=2, space="PSUM") as psc,
            tc.tile_pool(name="psq", bufs=2, space="PSUM") as psq,
            tc.tile_pool(name="psy", bufs=2, space="PSUM") as psy,
            tc.tile_pool(name="ynormp", bufs=4) as ynormp,
            tc.tile_pool(name="outp", bufs=3) as outp,
        ):
            mask_sb = constp.tile([128, 4, SB], bf16)
            bo_sb = constp.tile([128, NCH], f32)
            wo_sb = constp.tile([128, NP, NCH, 128], bf16)
            # scalar HWDGE ring: tiny consts now; masks/wo are issued
            # after the xt quarters so they never delay the first matmuls.
            nc.scalar.dma_start(bo_sb[:], bo_d[:])

            ynorm = []  # per-pair [128, S] bf16 normalized attention outputs

            with (
                tc.tile_pool(name="xtp", bufs=1) as xtp,
                tc.tile_pool(name="wp", bufs=2) as wp,
                tc.tile_pool(name="qkv", bufs=2) as qkvp,
                tc.tile_pool(name="vtp", bufs=2) as vtp,
                tc.tile_pool(name="vstp", bufs=2) as vstp,
                tc.tile_pool(name="vp", bufs=2) as vpool,
                tc.tile_pool(name="pp", bufs=4) as ppool,
                tc.tile_pool(name="yun", bufs=4) as yunp,
                tc.tile_pool(name="rp", bufs=2) as rpool,
                tc.tile_pool(name="rdp", bufs=4, space="DRAM") as rdpool,
            ):
                xt = xtp.tile([128, NCH, S], bf16)

                def alloc_pair(pp_):
                    qt_ = qkvp.tile([128, S], bf16, tag="qt", name="qt")
                    kt_ = qkvp.tile([128, S], bf16, tag="kt", name="kt")
                    vt_ = vtp.tile([128, S], bf16, tag="vt", name="vt")
                    # per (kb, h): 64 head dims + ones col at 64 (+1 pad
                    # so the h stride is 4B-aligned for fast DVE copies)
                    vs_ = vpool.tile([128, NKB, 2, 66], bf16, tag="v",
                                     name="v_sb")
                    vst_ = vstp.tile([128, NKB, 128], bf16, tag="vst",
                                     name="vst")
                    nc.vector.memset(vs_[:, :, :, 64:65], 1.0)
                    return qt_, kt_, vt_, vs_, vst_

                def emit_qkv(pp_, qt_, kt_, vt_, vs_, vst_):
                    """Yield small units of pair pp_'s QKV projection work.

                    V is projected first so its transpose (one xbar DMA
                    for the whole [128, S] tile) can start early.
                    """
                    w_tiles = []
                    for nm, wd in (("wv", wv_d), ("wq", wq_d), ("wk", wk_d)):
                        wt = wp.tile([128, NCH, 128], bf16, tag=nm, name=nm)
                        nc.sync.dma_start(wt[:], wd[pp_])
                        w_tiles.append(wt)
                    yield
                    for wi, (wt, dst) in enumerate(
                            zip(w_tiles, (vt_, qt_, kt_))):
                        for sblk in range(4):
                            acc = psq.tile([128, SB], f32, tag="acc",
                                           name="qacc")
                            for ci in range(NCH):
                                nc.tensor.matmul(
                                    acc[:],
                                    wt[:, ci, :],
                                    xt[:, ci, sblk * SB:(sblk + 1) * SB],
                                    start=(ci == 0),
                                    stop=(ci == NCH - 1),
                                )
                                if ci % 4 == 3:
                                    yield
                            nc.vector.tensor_copy(
                                dst[:, sblk * SB:(sblk + 1) * SB],
                                acc[:],
                            )
                            yield
                        if wi == 0:
                            # vt complete: one xbar transpose VT -> V
                            # (s = kb*128 + partition tiling), then per-kb
                            # DVE copies into the ones-augmented layout.
                            nc.sync.dma_start_transpose(vst_[:], vt_[:])
                            for kb in range(NKB):
                                nc.vector.tensor_copy(
                                    vs_[:, kb, :, 0:64],
                                    vst_[:, kb, :].rearrange(
                                        "k (h d) -> k h d", h=2),
                                )
                                if kb % 4 == 3:
                                    yield

                def emit_outproj(j):
                    """outT[:, j block] = Wo_g.T @ ynorm (+ bo/2), bf16."""
                    for g2 in range(4):
                        ot = outp.tile([128, 2, SB], bf16, tag="ot",
                                       name="ot")
                        for si in range(2):
                            dc = 2 * g2 + si
                            acc = psq.tile([128, SB], f32, tag="acc",
                                           name="oacc")
                            for pp in range(NP):
                                nc.tensor.matmul(
                                    acc[:],
                                    wo_sb[:, pp, dc, :],
                                    ynorm[pp][:, j * SB:(j + 1) * SB],
                                    start=(pp == 0),
                                    stop=(pp == NP - 1),
                                )
                                if pp % 2 == 1:
                                    yield
                            nc.vector.tensor_scalar_add(
                                ot[:, si, :],
                                acc[:],
                                bo_sb[:, dc:dc + 1],
                            )
                        nc.sync.dma_start(
                            out_dst[:, 2 * g2:2 * g2 + 2,
                                    j * SB:(j + 1) * SB],
                            ot[:],
                        )
                        yield

                pending = []

                def pump(n=1):
                    for _ in range(n):
                        while pending:
                            try:
                                next(pending[0])
                                break
                            except StopIteration:
                                pending.pop(0)
                        else:
                            break

                def drain():
                    while pending:
                        try:
                            next(pending[0])
                        except StopIteration:
                            pending.pop(0)

                # prologue: pair-0 weights on the sync ring first, then x
                cur = alloc_pair(0)
                g0 = emit_qkv(0, *cur)
                next(g0)  # issues the three weight DMAs
                for qtr in range(4):
                    eng = nc.sync if qtr % 2 == 0 else nc.scalar
                    eng.dma_start(
                        xt[:, :, qtr * SB:(qtr + 1) * SB],
                        xT_src[:, :, qtr * SB:(qtr + 1) * SB],
                    )
                nc.scalar.dma_start(mask_sb[:], mask_d[:])
                nc.scalar.dma_start(wo_sb[:], wo_d[:])
                for _ in g0:
                    pass

                def attn_j(p, j, qt, kt, vs_, yn):
                    """One q-block of causal attention for pair p.

                    Score blocks are processed two at a time so the four
                    row-tiled score matmuls pipeline with full 2-head
                    concurrency; PV lags one block-pair behind.
                    """
                    if True:
                        nkb_j = 4 * (j + 1)
                        yaccs = [
                            psy.tile([128, SB], f32, tag="y", name="yacc")
                            for _ in range(2)
                        ]
                        pend = []
                        for ki in range(nkb_j // 2):
                            newp = []
                            for kb in (2 * ki, 2 * ki + 1):
                                d = kb - 4 * j
                                qlo = max(0, d) * 128  # causal column trim
                                sc = psc.tile([128, 2, SB], f32, tag="sc",
                                              name="sc")
                                for h in range(2):
                                    hlo, hhi = h * 64, (h + 1) * 64
                                    nc.tensor.matmul(
                                        sc[:, h, qlo:],
                                        kt[hlo:hhi, kb * 128:(kb + 1) * 128],
                                        qt[hlo:hhi,
                                           j * SB + qlo:(j + 1) * SB],
                                        start=True,
                                        stop=True,
                                    )
                                newp.append((kb, sc, qlo))
                            for bi, (kb, sc, qlo) in enumerate(newp):
                                d = kb - 4 * j
                                pt = ppool.tile([128, 2, SB], bf16, tag="p",
                                                name="pt")
                                nc.scalar.activation(
                                    pt[:, :, qlo:],
                                    sc[:, :, qlo:],
                                    mybir.ActivationFunctionType.Exp,
                                    scale=0.125,
                                )
                                if d >= 0:
                                    for h in range(2):
                                        nc.vector.tensor_mul(
                                            pt[:, h, qlo:],
                                            pt[:, h, qlo:],
                                            mask_sb[:, d, qlo:],
                                        )
                                newp[bi] = (kb, pt, qlo)
                            for kb_, pt_, qlo_ in pend:
                                for h in range(2):
                                    nc.tensor.matmul(
                                        yaccs[h][0:65, qlo_:],
                                        vs_[:, kb_, h, 0:65],
                                        pt_[:, h, qlo_:],
                                        start=(kb_ == 0),
                                        stop=False,
                                    )
                            pend = newp
                            pump(3 if p == NP - 1 else 2)
                        for kb_, pt_, qlo_ in pend:
                            for h in range(2):
                                nc.tensor.matmul(
                                    yaccs[h][0:65, qlo_:],
                                    vs_[:, kb_, h, 0:65],
                                    pt_[:, h, qlo_:],
                                    start=(kb_ == 0),
                                    stop=(kb_ == nkb_j - 1),
                                )
                        # ---- per-j normalize: yn = yacc[0:64] / yacc[64].
                        # The reciprocal row is broadcast across partitions
                        # via a DRAM round-trip (both DMAs on the sync ring,
                        # whose FIFO order guarantees write-before-read).
                        y_uns = []
                        rbs = []
                        for h in range(2):
                            y_un = yunp.tile([65, SB], f32, tag="yun",
                                             name="y_un")
                            nc.vector.tensor_copy(y_un[:], yaccs[h][0:65, :])
                            rt = rpool.tile([1, SB], f32, tag="r",
                                            name="rt")
                            # custom-DVE recip only works at base
                            # partition 0: cross-partition copy first
                            nc.vector.tensor_copy(rt[:], y_un[64:65, :])
                            nc.vector.reciprocal_approx_fast(rt[:], rt[:])
                            rsc = rdpool.tile([1, SB], f32, tag="rsc",
                                              name="rsc")
                            nc.sync.dma_start(rsc[:], rt[:])
                            rb = rpool.tile([64, SB], f32, tag="rb",
                                            name="rb")
                            nc.sync.dma_start(
                                rb[:], rsc.to_broadcast([64, SB]))
                            y_uns.append(y_un)
                            rbs.append(rb)
                        for h in range(2):
                            hlo, hhi = h * 64, (h + 1) * 64
                            nc.vector.tensor_mul(
                                yn[hlo:hhi, j * SB:(j + 1) * SB],
                                y_uns[h][0:64, :],
                                rbs[h][:],
                            )
                        pump()

                for p in range(NP):
                    qt, kt, vt, vs_, vst_ = cur
                    if p + 1 < NP:
                        nxt = alloc_pair(p + 1)
                        pending.append(emit_qkv(p + 1, *nxt))
                    else:
                        nxt = None
                    yn = ynormp.tile([128, S], bf16, tag="yn", name="yn")
                    ynorm.append(yn)
                    for j in range(NSB):
                        attn_j(p, j, qt, kt, vs_, yn)
                        if p == NP - 1:
                            pending.append(emit_outproj(j))
                            pump()
                    drain()
                    cur = nxt

    nc.compile()
    return nc


def _get_program():
    if "nc" not in _CACHE:
        _CACHE["nc"] = _build_program()
    return _CACHE["nc"]


def kernel(x, Wq, Wk, Wv, Wo, bo):
    global LAST_RESULTS
    x = np.asarray(x, dtype=np.float32)
    Wq = np.asarray(Wq, dtype=np.float32)
    Wk = np.asarray(Wk, dtype=np.float32)
    Wv = np.asarray(Wv, dtype=np.float32)
    Wo = np.asarray(Wo, dtype=np.float32)
    bo = np.asarray(bo, dtype=np.float32)

    nc = _get_program()

    # constants shared by all cores
    masks = np.zeros((128, 4, SB), dtype=ml_dtypes.bfloat16)
    kk = np.arange(128)[:, None]
    qq = np.arange(SB)[None, :]
    for d in range(4):
        masks[:, d, :] = (128 * d + kk <= qq).astype(ml_dtypes.bfloat16)
    bo2 = np.ascontiguousarray((bo / 2.0).reshape(NCH, 128).T)

    def wshape(w):  # [D, DG] -> [NP, 128, NCH, 128] bf16
        return np.ascontiguousarray(
            w.reshape(NCH, 128, NP, 128).transpose(2, 1, 0, 3)
        ).astype(ml_dtypes.bfloat16)

    in_maps = []
    for c in range(8):
        b, g = c // 2, c % 2
        xT = np.ascontiguousarray(x[b].T).astype(ml_dtypes.bfloat16)
        wo_g = Wo[g * DG:(g + 1) * DG, :]
        wo_dev = np.ascontiguousarray(
            wo_g.reshape(NP, 128, NCH, 128).transpose(1, 0, 2, 3)
        ).astype(ml_dtypes.bfloat16)
        in_maps.append({
            "xT": xT,
            "Wq": wshape(Wq[:, g * DG:(g + 1) * DG]),
            "Wk": wshape(Wk[:, g * DG:(g + 1) * DG]),
            "Wv": wshape(Wv[:, g * DG:(g + 1) * DG]),
            "Wo": wo_dev,
            "bo2": bo2,
            "masks": masks,
        })

    trace = bool(os.environ.get("BASS_TRACE"))
    if trace:
        _install_trace_shim()
    res = None
    for attempt in range(3):
        try:
            res = bass_utils.run_bass_kernel_spmd(
                nc, in_maps, core_ids=list(range(8)), trace=trace)
            break
        except Exception:
            if attempt == 2:
                raise
    LAST_RESULTS = res

    out = np.empty((B, S, D), dtype=np.float32)
    for b in range(B):
        acc = (res.results[2 * b]["outT"].astype(np.float32)
               + res.results[2 * b + 1]["outT"].astype(np.float32))
        out[b] = acc.T
    return out
